# revision 24
# baseline (speedup 1.0000x reference)
"""Multi-head causal attention (B=4, T=2048, C=1024, H=16) on 8 trn2 cores.

Sharding: core = (batch b, head-half hg): each core computes QKV for batch b
and its 8 heads, causal flash-style attention (scores kept transposed
[key, query] so softmax denominators come from an appended ones-column in V),
and a partial output projection over its 512 y-features. Host sums the two
partial projections per batch (pure unshard-reduce; bias folded into the
hg==0 core's projection). No cross-core communication on device.

v2: phases are woven (P1 nt / P2 qt / P3 deferred) so the Tile scheduler
overlaps the scalar-engine exp backlog of attention with the QKV / output
projection matmuls; scalar engine runs exp only; diagonal score blocks are
query-range-restricted; output staged in bf16.
"""

import numpy as np
import ml_dtypes
import concourse.bass as bass
import concourse.mybir as mybir
import concourse.tile as tile
from concourse import bacc
from concourse.bass_utils import run_bass_kernel_spmd

B, T, C = 4, 2048, 1024
H, D = 16, 64
F32 = mybir.dt.float32
F32R = mybir.dt.float32r
BF16 = mybir.dt.bfloat16
AFT = mybir.ActivationFunctionType

_CACHE = {}


def build():
    nc = bacc.Bacc(None, target_bir_lowering=False)
    xt_d = nc.dram_tensor("xt", [C, T], BF16, kind="ExternalInput")
    wq_d = nc.dram_tensor("wq", [C, 512], BF16, kind="ExternalInput")
    wk_d = nc.dram_tensor("wk", [C, 512], BF16, kind="ExternalInput")
    wv_d = nc.dram_tensor("wv", [C, 512], BF16, kind="ExternalInput")
    bqk_d = nc.dram_tensor("bqk", [128, 8], F32, kind="ExternalInput")
    bvr_d = nc.dram_tensor("bvr", [128, 512], BF16, kind="ExternalInput")
    bpr_d = nc.dram_tensor("bpr", [128, C], BF16, kind="ExternalInput")
    masks_d = nc.dram_tensor("masks", [128, 896], BF16, kind="ExternalInput")
    wp_d = nc.dram_tensor("wp", [512, C], BF16, kind="ExternalInput")
    out_d = nc.dram_tensor("out", [T, C], BF16, kind="ExternalOutput")

    with nc.allow_low_precision(reason="fp32r matmul pipeline"):
        with tile.TileContext(nc) as tc:
            with (
                tc.tile_pool(name="const", bufs=1) as constp,
                tc.tile_pool(name="qk", bufs=1) as qkp,
                tc.tile_pool(name="vpool", bufs=1) as vp,
                tc.tile_pool(name="esb", bufs=2) as ep,
                tc.tile_pool(name="small", bufs=2) as smallp,
                tc.tile_pool(name="p1w", bufs=1) as p1wp,
                tc.tile_pool(name="p1x", bufs=1) as p1xp,
                tc.tile_pool(name="p3w", bufs=1) as p3wp,
                tc.tile_pool(name="ps", bufs=2, space="PSUM") as psp,
            ):
                bqk_t = constp.tile([128, 8], F32, tag="bqk")
                nc.sync.dma_start(bqk_t[:], bqk_d[:])
                maskE = constp.tile([128, 896], BF16, tag="maskE")
                nc.sync.dma_start(maskE[:], masks_d[:])

                qT = [qkp.tile([128, T], BF16, tag=f"qT{j}", name=f"qT{j}") for j in range(4)]
                kT = [qkp.tile([128, T], BF16, tag=f"kT{j}", name=f"kT{j}") for j in range(4)]
                vS = [vp.tile([128, 520], BF16, tag=f"v{t}", name=f"v{t}") for t in range(16)]
                yT = [qkp.tile([128, T], BF16, tag=f"yT{j}", name=f"yT{j}") for j in range(4)]

                # The whole startup is HBM-BW-bound and DMA-issue-rate-bound,
                # so each tensor is loaded with ONE multi-dim DMA, spread
                # across the three DMA-capable queues (sync/scalar/gpsimd),
                # x + wq (the first accumulation group's inputs) first.
                def load_x(nt, eng):
                    ts0 = nt * 512
                    xx = p1xp.tile([128, 4096], BF16, tag="xsb", bufs=2)
                    eng.dma_start(
                        xx[:].rearrange("p (c t) -> p c t", t=512),
                        xt_d[:, ts0:ts0 + 512].rearrange("(c p) t -> p c t", p=128))
                    return xx

                xt0 = load_x(0, nc.sync)

                wq_sb = p1wp.tile([128, 4096], BF16, tag="wq")
                nc.gpsimd.dma_start(wq_sb[:].rearrange("p (c f) -> p c f", f=512),
                                    wq_d[:, :].rearrange("(c p) f -> p c f", p=128))
                wk_sb = p1wp.tile([128, 4096], BF16, tag="wk")
                nc.scalar.dma_start(wk_sb[:].rearrange("p (c f) -> p c f", f=512),
                                    wk_d[:, :].rearrange("(c p) f -> p c f", p=128))
                wv_sb = p1wp.tile([128, 4096], BF16, tag="wv")
                nc.sync.dma_start(wv_sb[:].rearrange("p (c f) -> p c f", f=512),
                                  wv_d[:, :].rearrange("(c p) f -> p c f", p=128))
                bvr_t = p1wp.tile([128, 512], BF16, tag="bvr")
                nc.sync.dma_start(bvr_t[:], bvr_d[:])
                wp_sb = p3wp.tile([128, 4096], BF16, tag="wp")
                bpr_t = p3wp.tile([128, C], BF16, tag="bpr")
                onesc = constp.tile([1, 64], F32, tag="onesc")
                nc.vector.memset(onesc[:], 1.0)

                # ---------------- P1: QKV projections for token block nt ----
                def p1(nt, xt=None):
                    ts0 = nt * 512
                    if xt is None:
                        xt = load_x(nt, nc.sync if nt % 2 == 0 else nc.gpsimd)

                    def qk_group(ft):
                        q_ps = psp.tile([128, 512], F32, tag="mmps")
                        for c in range(8):
                            nc.tensor.matmul(q_ps[:],
                                             wq_sb[:, c * 512 + ft * 128:c * 512 + ft * 128 + 128],
                                             xt[:, c * 512:(c + 1) * 512],
                                             start=(c == 0), stop=(c == 7))
                        nc.vector.tensor_scalar_add(qT[ft][:, ts0:ts0 + 512], q_ps[:],
                                                    bqk_t[:, ft:ft + 1])
                        k_ps = psp.tile([128, 512], F32, tag="mmps")
                        for c in range(8):
                            nc.tensor.matmul(k_ps[:],
                                             wk_sb[:, c * 512 + ft * 128:c * 512 + ft * 128 + 128],
                                             xt[:, c * 512:(c + 1) * 512],
                                             start=(c == 0), stop=(c == 7))
                        nc.vector.tensor_scalar_add(kT[ft][:, ts0:ts0 + 512], k_ps[:],
                                                    bqk_t[:, 4 + ft:5 + ft])

                    def v_group(t2):
                        tt = nt * 4 + t2
                        v_ps = psp.tile([128, 512], F32, tag="mmps")
                        for c in range(8):
                            nc.tensor.matmul(v_ps[:],
                                             xt[:, c * 512 + t2 * 128:c * 512 + t2 * 128 + 128],
                                             wv_sb[:, c * 512:(c + 1) * 512],
                                             start=(c == 0), stop=(c == 7))
                        vv = vS[tt][:].rearrange("p (h c) -> p h c", c=65)
                        nc.vector.tensor_add(vv[:, :, 0:64],
                                             v_ps[:].rearrange("p (h c) -> p h c", c=64),
                                             bvr_t[:].rearrange("p (h c) -> p h c", c=64))
                        nc.vector.memset(vv[:, :, 64:65], 1.0)

                    if nt == 0:
                        # q/k head-pair 0 first, then all v, so attention on
                        # the first query block can start ASAP
                        qk_group(0)
                        for t2 in range(4):
                            v_group(t2)
                        for ft in range(1, 4):
                            qk_group(ft)
                    else:
                        for ft in range(4):
                            qk_group(ft)
                        for t2 in range(4):
                            v_group(t2)

                # ---------------- P2: attention for query block qt ----------
                def p2(qt):
                    q0 = qt * 512
                    ext = 4 * (qt + 1)
                    coll2 = [smallp.tile([8, 512], F32, tag=f"coll{a}", bufs=2,
                                         name=f"coll{a}") for a in range(2 if qt == 3 else 1)]
                    ysbs = []
                    for pj in range(4):
                        y_ps = [psp.tile([65, 512], F32, tag=f"yps{h}", bufs=1,
                                         name=f"yps{h}") for h in range(2)]
                        for sc in range(ext):
                            r = sc - (ext - 4)
                            qlo = 128 * r if r >= 2 else 0
                            s_ps = psp.tile([128, 1024], F32, tag="sps")
                            nc.tensor.matmul(s_ps[:, qlo:512],
                                             kT[pj][0:64, sc * 128:(sc + 1) * 128],
                                             qT[pj][0:64, q0 + qlo:q0 + 512],
                                             start=True, stop=True, tile_position=(0, 0))
                            nc.tensor.matmul(s_ps[:, 512 + qlo:1024],
                                             kT[pj][64:128, sc * 128:(sc + 1) * 128],
                                             qT[pj][64:128, q0 + qlo:q0 + 512],
                                             start=True, stop=True, tile_position=(64, 0))
                            e_t = ep.tile([128, 1024], BF16, tag="e", bufs=6)
                            if qlo == 0:
                                nc.scalar.activation(e_t[:], s_ps[:], AFT.Exp, scale=0.125)
                            else:
                                for h in range(2):
                                    nc.scalar.activation(
                                        e_t[:, 512 * h + qlo:512 * h + 512],
                                        s_ps[:, 512 * h + qlo:512 * h + 512],
                                        AFT.Exp, scale=0.125)
                            if r >= 0:
                                m0 = 384 - 128 * r
                                for h in range(2):
                                    nc.vector.tensor_mul(
                                        e_t[:, 512 * h + qlo:512 * h + 512],
                                        e_t[:, 512 * h + qlo:512 * h + 512],
                                        maskE[:, m0 + qlo:m0 + 512])
                            for h in range(2):
                                hc = 130 * pj + 65 * h
                                nc.tensor.matmul(y_ps[h][:, qlo:512], vS[sc][:, hc:hc + 65],
                                                 e_t[:, 512 * h + qlo:512 * h + 512],
                                                 start=(sc == 0), stop=(sc == ext - 1))
                        for h in range(2):
                            i = 2 * pj + h
                            y_sb = smallp.tile([65, 512], F32, tag="ysb", bufs=12)
                            nc.vector.tensor_copy(y_sb[:], y_ps[h][:])
                            if qt == 3 and pj >= 2:
                                nc.sync.dma_start(coll2[1][i - 4:i - 3, :], y_sb[64:65, :])
                            else:
                                nc.sync.dma_start(coll2[0][i:i + 1, :], y_sb[64:65, :])
                            ysbs.append(y_sb)
                    halves = [(0, 2, 0), (2, 4, 1)] if qt == 3 else [(0, 4, 0)]
                    for (plo, phi, a) in halves:
                        n2 = 2 * (phi - plo)
                        rec8 = smallp.tile([8, 512], F32, tag="rec8", bufs=2)
                        nc.vector.reciprocal_approx_fast(rec8[0:n2, :], coll2[a][0:n2, :])
                        for pj in range(plo, phi):
                            for h in range(2):
                                i = 2 * pj + h
                                i0 = i - 4 * a
                                r_t = smallp.tile([1, 512], F32, tag="rt", bufs=4)
                                nc.sync.dma_start(r_t[:], rec8[i0:i0 + 1, :])
                                if qt == 3:
                                    # tail-critical: broadcast across partitions
                                    # via a rank-1 matmul (tensor is starving
                                    # here), not the slow gpsimd path
                                    rb_ps = psp.tile([64, 512], F32, tag="mmps")
                                    nc.tensor.matmul(rb_ps[:], onesc[:, :].bitcast(F32R),
                                                     r_t[:].bitcast(F32R),
                                                     start=True, stop=True)
                                    nc.vector.tensor_mul(
                                        yT[pj][64 * h:64 * h + 64, q0:q0 + 512],
                                        ysbs[i][0:64, :], rb_ps[:])
                                else:
                                    rb_t = smallp.tile([64, 512], F32, tag="rbt", bufs=4)
                                    nc.gpsimd.partition_broadcast(rb_t[:], r_t[:])
                                    nc.vector.tensor_mul(
                                        yT[pj][64 * h:64 * h + 64, q0:q0 + 512],
                                        ysbs[i][0:64, :], rb_t[:])

                # ---------------- P3: output projection for query block qt --
                def p3(qt):
                    for t2 in range(4):
                        tt = 4 * qt + t2
                        o_t = smallp.tile([128, 1024], BF16, tag="osb", bufs=3)
                        for of in range(2):
                            o_ps = psp.tile([128, 512], F32, tag="mmps")
                            for cy in range(4):
                                nc.tensor.matmul(
                                    o_ps[:],
                                    yT[cy][:, tt * 128:(tt + 1) * 128],
                                    wp_sb[:, cy * 1024 + of * 512:cy * 1024 + (of + 1) * 512],
                                    start=(cy == 0), stop=(cy == 3))
                            nc.vector.tensor_add(o_t[:, of * 512:(of + 1) * 512], o_ps[:],
                                                 bpr_t[:, of * 512:(of + 1) * 512])
                        nc.gpsimd.dma_start(out_d[tt * 128:(tt + 1) * 128, :], o_t[:])

                p1(0, xt0)
                nc.gpsimd.dma_start(wp_sb[:].rearrange("p (c f) -> p c f", f=1024),
                                    wp_d[:, :].rearrange("(c p) f -> p c f", p=128))
                nc.sync.dma_start(bpr_t[:], bpr_d[:])
                for qt in range(4):
                    p2(qt)
                    if qt < 3:
                        p1(qt + 1)
                for qt in range(4):
                    p3(qt)

    if not nc.is_finalized():
        nc.finalize()
    return nc


def _get_nc():
    if "nc" not in _CACHE:
        _CACHE["nc"] = build()
    return _CACHE["nc"]


def _masks():
    i = np.arange(128)[:, None]
    x = np.arange(896)[None, :] - 384
    return np.where(i <= x, 1.0, 0.0).astype(ml_dtypes.bfloat16)


def kernel(x, w_attn, b_attn, w_proj, b_proj, _trace=False, _trace_kwargs=None):
    x = np.asarray(x, dtype=np.float32)
    w_attn = np.asarray(w_attn, dtype=np.float32)
    b_attn = np.asarray(b_attn, dtype=np.float32)
    w_proj = np.asarray(w_proj, dtype=np.float32)
    b_proj = np.asarray(b_proj, dtype=np.float32)

    masks = _masks()
    in_maps = []
    for core in range(8):
        b, hg = core // 2, core % 2
        cs = hg * 512
        bq = b_attn[cs:cs + 512]
        bk = b_attn[C + cs:C + cs + 512]
        bqk = np.concatenate([bq.reshape(4, 128).T, bk.reshape(4, 128).T],
                             axis=1).astype(np.float32)
        wpb = b_proj if hg == 0 else np.zeros_like(b_proj)
        bf = ml_dtypes.bfloat16
        bv = b_attn[2 * C + cs:2 * C + cs + 512]
        in_maps.append({
            "xt": np.ascontiguousarray(x[b].T).astype(bf),
            "wq": np.ascontiguousarray(w_attn[:, cs:cs + 512]).astype(bf),
            "wk": np.ascontiguousarray(w_attn[:, C + cs:C + cs + 512]).astype(bf),
            "wv": np.ascontiguousarray(w_attn[:, 2 * C + cs:2 * C + cs + 512]).astype(bf),
            "bqk": bqk,
            "bvr": np.ascontiguousarray(np.broadcast_to(bv[None, :], (128, 512))).astype(bf),
            "bpr": np.ascontiguousarray(np.broadcast_to(wpb[None, :], (128, C))).astype(bf),
            "masks": masks,
            "wp": np.ascontiguousarray(w_proj[cs:cs + 512, :]).astype(bf),
        })

    kw = {}
    if _trace:
        kw["trace"] = True
        if _trace_kwargs:
            kw.update(_trace_kwargs)
    res = run_bass_kernel_spmd(_get_nc(), in_maps, list(range(8)), **kw)
    _CACHE["last_results"] = res
    outs = [np.asarray(res.results[c]["out"], dtype=np.float32) for c in range(8)]
    y = np.stack([outs[2 * b] + outs[2 * b + 1] for b in range(B)])
    return y.astype(np.float32)


# revision 27
# speedup vs baseline: 1.0028x; 1.0028x over previous
"""Multi-head causal attention (B=4, T=2048, C=1024, H=16) on 8 trn2 cores.

Sharding: core = (batch b, head-half hg): each core computes QKV for batch b
and its 8 heads, causal flash-style attention (scores kept transposed
[key, query] so softmax denominators come from an appended ones-column in V),
and a partial output projection over its 512 y-features. Host sums the two
partial projections per batch (pure unshard-reduce; bias folded into the
hg==0 core's projection). No cross-core communication on device.

v2: phases are woven (P1 nt / P2 qt / P3 deferred) so the Tile scheduler
overlaps the scalar-engine exp backlog of attention with the QKV / output
projection matmuls; scalar engine runs exp only; diagonal score blocks are
query-range-restricted; output staged in bf16.
"""

import numpy as np
import ml_dtypes
import concourse.bass as bass
import concourse.mybir as mybir
import concourse.tile as tile
from concourse import bacc
from concourse.bass_utils import run_bass_kernel_spmd

B, T, C = 4, 2048, 1024
H, D = 16, 64
F32 = mybir.dt.float32
F32R = mybir.dt.float32r
BF16 = mybir.dt.bfloat16
AFT = mybir.ActivationFunctionType

_CACHE = {}


def build():
    nc = bacc.Bacc(None, target_bir_lowering=False)
    xt_d = nc.dram_tensor("xt", [C, T], BF16, kind="ExternalInput")
    wq_d = nc.dram_tensor("wq", [C, 512], BF16, kind="ExternalInput")
    wk_d = nc.dram_tensor("wk", [C, 512], BF16, kind="ExternalInput")
    wv_d = nc.dram_tensor("wv", [C, 512], BF16, kind="ExternalInput")
    bqk_d = nc.dram_tensor("bqk", [128, 8], F32, kind="ExternalInput")
    bvr_d = nc.dram_tensor("bvr", [128, 512], BF16, kind="ExternalInput")
    bpr_d = nc.dram_tensor("bpr", [128, C], BF16, kind="ExternalInput")
    masks_d = nc.dram_tensor("masks", [128, 896], BF16, kind="ExternalInput")
    wp_d = nc.dram_tensor("wp", [512, C], BF16, kind="ExternalInput")
    out_d = nc.dram_tensor("out", [T, C], BF16, kind="ExternalOutput")

    with nc.allow_low_precision(reason="fp32r matmul pipeline"):
        with tile.TileContext(nc) as tc:
            with (
                tc.tile_pool(name="const", bufs=1) as constp,
                tc.tile_pool(name="qk", bufs=1) as qkp,
                tc.tile_pool(name="vpool", bufs=1) as vp,
                tc.tile_pool(name="esb", bufs=2) as ep,
                tc.tile_pool(name="small", bufs=2) as smallp,
                tc.tile_pool(name="p1w", bufs=1) as p1wp,
                tc.tile_pool(name="p1x", bufs=1) as p1xp,
                tc.tile_pool(name="p3w", bufs=1) as p3wp,
                tc.tile_pool(name="ps", bufs=2, space="PSUM") as psp,
            ):
                bqk_t = constp.tile([128, 8], F32, tag="bqk")
                nc.sync.dma_start(bqk_t[:], bqk_d[:])
                maskE = constp.tile([128, 896], BF16, tag="maskE")
                nc.sync.dma_start(maskE[:], masks_d[:])

                qT = [qkp.tile([128, T], BF16, tag=f"qT{j}", name=f"qT{j}") for j in range(4)]
                kT = [qkp.tile([128, T], BF16, tag=f"kT{j}", name=f"kT{j}") for j in range(4)]
                vS = [vp.tile([128, 520], BF16, tag=f"v{t}", name=f"v{t}") for t in range(16)]
                yT = [qkp.tile([128, T], BF16, tag=f"yT{j}", name=f"yT{j}") for j in range(4)]

                # The whole startup is HBM-BW-bound and DMA-issue-rate-bound,
                # so each tensor is loaded with ONE multi-dim DMA, spread
                # across the three DMA-capable queues (sync/scalar/gpsimd),
                # x + wq (the first accumulation group's inputs) first.
                # Per-c 128-row DMAs (cheap ~600ns issues) into merged
                # tiles; nt0's x and wq are interleaved first so the first
                # accumulation group's c=0 inputs land ASAP.
                def load_x(nt, engs):
                    ts0 = nt * 512
                    xx = p1xp.tile([128, 4096], BF16, tag="xsb", bufs=2)
                    for c in range(8):
                        engs[c % len(engs)].dma_start(
                            xx[:, c * 512:(c + 1) * 512],
                            xt_d[c * 128:(c + 1) * 128, ts0:ts0 + 512])
                    return xx

                wq_sb = p1wp.tile([128, 4096], BF16, tag="wq")
                wk_sb = p1wp.tile([128, 4096], BF16, tag="wk")
                wv_sb = p1wp.tile([128, 4096], BF16, tag="wv")
                xt0 = p1xp.tile([128, 4096], BF16, tag="xsb", bufs=2)
                for c in range(8):
                    (nc.sync if c % 2 == 0 else nc.gpsimd).dma_start(
                        xt0[:, c * 512:(c + 1) * 512],
                        xt_d[c * 128:(c + 1) * 128, 0:512])
                    nc.scalar.dma_start(wq_sb[:, c * 512:(c + 1) * 512],
                                        wq_d[c * 128:(c + 1) * 128, :])
                for c in range(8):
                    (nc.gpsimd if c % 2 == 0 else nc.sync).dma_start(
                        wk_sb[:, c * 512:(c + 1) * 512],
                        wk_d[c * 128:(c + 1) * 128, :])
                for c in range(8):
                    (nc.sync if c % 2 == 0 else nc.gpsimd).dma_start(
                        wv_sb[:, c * 512:(c + 1) * 512],
                        wv_d[c * 128:(c + 1) * 128, :])
                bvr_t = p1wp.tile([128, 512], BF16, tag="bvr")
                nc.sync.dma_start(bvr_t[:], bvr_d[:])
                wp_sb = p3wp.tile([128, 4096], BF16, tag="wp")
                bpr_t = p3wp.tile([128, C], BF16, tag="bpr")
                onesc = constp.tile([1, 64], F32, tag="onesc")
                nc.vector.memset(onesc[:], 1.0)

                # ---------------- P1: QKV projections for token block nt ----
                def p1(nt, xt=None):
                    ts0 = nt * 512
                    if xt is None:
                        xt = load_x(nt, [nc.sync, nc.gpsimd])

                    def qk_group(ft):
                        q_ps = psp.tile([128, 512], F32, tag="mmps")
                        for c in range(8):
                            nc.tensor.matmul(q_ps[:],
                                             wq_sb[:, c * 512 + ft * 128:c * 512 + ft * 128 + 128],
                                             xt[:, c * 512:(c + 1) * 512],
                                             start=(c == 0), stop=(c == 7))
                        nc.vector.tensor_scalar_add(qT[ft][:, ts0:ts0 + 512], q_ps[:],
                                                    bqk_t[:, ft:ft + 1])
                        k_ps = psp.tile([128, 512], F32, tag="mmps")
                        for c in range(8):
                            nc.tensor.matmul(k_ps[:],
                                             wk_sb[:, c * 512 + ft * 128:c * 512 + ft * 128 + 128],
                                             xt[:, c * 512:(c + 1) * 512],
                                             start=(c == 0), stop=(c == 7))
                        nc.vector.tensor_scalar_add(kT[ft][:, ts0:ts0 + 512], k_ps[:],
                                                    bqk_t[:, 4 + ft:5 + ft])

                    def v_group(t2):
                        tt = nt * 4 + t2
                        v_ps = psp.tile([128, 512], F32, tag="mmps")
                        for c in range(8):
                            nc.tensor.matmul(v_ps[:],
                                             xt[:, c * 512 + t2 * 128:c * 512 + t2 * 128 + 128],
                                             wv_sb[:, c * 512:(c + 1) * 512],
                                             start=(c == 0), stop=(c == 7))
                        vv = vS[tt][:].rearrange("p (h c) -> p h c", c=65)
                        nc.vector.tensor_add(vv[:, :, 0:64],
                                             v_ps[:].rearrange("p (h c) -> p h c", c=64),
                                             bvr_t[:].rearrange("p (h c) -> p h c", c=64))
                        nc.vector.memset(vv[:, :, 64:65], 1.0)

                    if nt == 0:
                        # q/k head-pair 0 first, then all v, so attention on
                        # the first query block can start ASAP
                        qk_group(0)
                        for t2 in range(4):
                            v_group(t2)
                        for ft in range(1, 4):
                            qk_group(ft)
                    else:
                        for ft in range(4):
                            qk_group(ft)
                        for t2 in range(4):
                            v_group(t2)

                # ---------------- P2: attention for query block qt ----------
                def p2(qt):
                    q0 = qt * 512
                    ext = 4 * (qt + 1)
                    coll2 = [smallp.tile([8, 512], F32, tag=f"coll{a}", bufs=2,
                                         name=f"coll{a}") for a in range(2 if qt == 3 else 1)]
                    ysbs = []
                    for pj in range(4):
                        y_ps = [psp.tile([65, 512], F32, tag=f"yps{h}", bufs=1,
                                         name=f"yps{h}") for h in range(2)]
                        for sc in range(ext):
                            r = sc - (ext - 4)
                            qlo = 128 * r if r >= 2 else 0
                            s_ps = psp.tile([128, 1024], F32, tag="sps")
                            nc.tensor.matmul(s_ps[:, qlo:512],
                                             kT[pj][0:64, sc * 128:(sc + 1) * 128],
                                             qT[pj][0:64, q0 + qlo:q0 + 512],
                                             start=True, stop=True, tile_position=(0, 0))
                            nc.tensor.matmul(s_ps[:, 512 + qlo:1024],
                                             kT[pj][64:128, sc * 128:(sc + 1) * 128],
                                             qT[pj][64:128, q0 + qlo:q0 + 512],
                                             start=True, stop=True, tile_position=(64, 0))
                            e_t = ep.tile([128, 1024], BF16, tag="e", bufs=6)
                            if qlo == 0:
                                nc.scalar.activation(e_t[:], s_ps[:], AFT.Exp, scale=0.125)
                            else:
                                for h in range(2):
                                    nc.scalar.activation(
                                        e_t[:, 512 * h + qlo:512 * h + 512],
                                        s_ps[:, 512 * h + qlo:512 * h + 512],
                                        AFT.Exp, scale=0.125)
                            if r >= 0:
                                m0 = 384 - 128 * r
                                for h in range(2):
                                    nc.vector.tensor_mul(
                                        e_t[:, 512 * h + qlo:512 * h + 512],
                                        e_t[:, 512 * h + qlo:512 * h + 512],
                                        maskE[:, m0 + qlo:m0 + 512])
                            for h in range(2):
                                hc = 130 * pj + 65 * h
                                nc.tensor.matmul(y_ps[h][:, qlo:512], vS[sc][:, hc:hc + 65],
                                                 e_t[:, 512 * h + qlo:512 * h + 512],
                                                 start=(sc == 0), stop=(sc == ext - 1))
                        for h in range(2):
                            i = 2 * pj + h
                            y_sb = smallp.tile([65, 512], F32, tag="ysb", bufs=12)
                            nc.vector.tensor_copy(y_sb[:], y_ps[h][:])
                            if qt == 3 and pj >= 2:
                                nc.sync.dma_start(coll2[1][i - 4:i - 3, :], y_sb[64:65, :])
                            else:
                                nc.sync.dma_start(coll2[0][i:i + 1, :], y_sb[64:65, :])
                            ysbs.append(y_sb)
                    halves = [(0, 2, 0), (2, 4, 1)] if qt == 3 else [(0, 4, 0)]
                    for (plo, phi, a) in halves:
                        n2 = 2 * (phi - plo)
                        rec8 = smallp.tile([8, 512], F32, tag="rec8", bufs=2)
                        nc.vector.reciprocal_approx_fast(rec8[0:n2, :], coll2[a][0:n2, :])
                        for pj in range(plo, phi):
                            for h in range(2):
                                i = 2 * pj + h
                                i0 = i - 4 * a
                                r_t = smallp.tile([1, 512], F32, tag="rt", bufs=4)
                                nc.sync.dma_start(r_t[:], rec8[i0:i0 + 1, :])
                                if qt == 3:
                                    # tail-critical: broadcast across partitions
                                    # via a rank-1 matmul (tensor is starving
                                    # here), not the slow gpsimd path
                                    rb_ps = psp.tile([64, 512], F32, tag="mmps")
                                    nc.tensor.matmul(rb_ps[:], onesc[:, :].bitcast(F32R),
                                                     r_t[:].bitcast(F32R),
                                                     start=True, stop=True)
                                    nc.vector.tensor_mul(
                                        yT[pj][64 * h:64 * h + 64, q0:q0 + 512],
                                        ysbs[i][0:64, :], rb_ps[:])
                                else:
                                    rb_t = smallp.tile([64, 512], F32, tag="rbt", bufs=4)
                                    nc.gpsimd.partition_broadcast(rb_t[:], r_t[:])
                                    nc.vector.tensor_mul(
                                        yT[pj][64 * h:64 * h + 64, q0:q0 + 512],
                                        ysbs[i][0:64, :], rb_t[:])

                # ---------------- P3: output projection for query block qt --
                def p3(qt):
                    for t2 in range(4):
                        tt = 4 * qt + t2
                        o_t = smallp.tile([128, 1024], BF16, tag="osb", bufs=3)
                        for of in range(2):
                            o_ps = psp.tile([128, 512], F32, tag="mmps")
                            for cy in range(4):
                                nc.tensor.matmul(
                                    o_ps[:],
                                    yT[cy][:, tt * 128:(tt + 1) * 128],
                                    wp_sb[:, cy * 1024 + of * 512:cy * 1024 + (of + 1) * 512],
                                    start=(cy == 0), stop=(cy == 3))
                            nc.vector.tensor_add(o_t[:, of * 512:(of + 1) * 512], o_ps[:],
                                                 bpr_t[:, of * 512:(of + 1) * 512])
                        nc.gpsimd.dma_start(out_d[tt * 128:(tt + 1) * 128, :], o_t[:])

                p1(0, xt0)
                for c in range(4):
                    nc.gpsimd.dma_start(wp_sb[:, c * 1024:(c + 1) * 1024],
                                        wp_d[c * 128:(c + 1) * 128, :])
                nc.sync.dma_start(bpr_t[:], bpr_d[:])
                for qt in range(4):
                    p2(qt)
                    if qt < 3:
                        p1(qt + 1)
                for qt in range(4):
                    p3(qt)

    if not nc.is_finalized():
        nc.finalize()
    return nc


def _get_nc():
    if "nc" not in _CACHE:
        _CACHE["nc"] = build()
    return _CACHE["nc"]


def _masks():
    i = np.arange(128)[:, None]
    x = np.arange(896)[None, :] - 384
    return np.where(i <= x, 1.0, 0.0).astype(ml_dtypes.bfloat16)


def kernel(x, w_attn, b_attn, w_proj, b_proj, _trace=False, _trace_kwargs=None):
    x = np.asarray(x, dtype=np.float32)
    w_attn = np.asarray(w_attn, dtype=np.float32)
    b_attn = np.asarray(b_attn, dtype=np.float32)
    w_proj = np.asarray(w_proj, dtype=np.float32)
    b_proj = np.asarray(b_proj, dtype=np.float32)

    masks = _masks()
    in_maps = []
    for core in range(8):
        b, hg = core // 2, core % 2
        cs = hg * 512
        bq = b_attn[cs:cs + 512]
        bk = b_attn[C + cs:C + cs + 512]
        bqk = np.concatenate([bq.reshape(4, 128).T, bk.reshape(4, 128).T],
                             axis=1).astype(np.float32)
        wpb = b_proj if hg == 0 else np.zeros_like(b_proj)
        bf = ml_dtypes.bfloat16
        bv = b_attn[2 * C + cs:2 * C + cs + 512]
        in_maps.append({
            "xt": np.ascontiguousarray(x[b].T).astype(bf),
            "wq": np.ascontiguousarray(w_attn[:, cs:cs + 512]).astype(bf),
            "wk": np.ascontiguousarray(w_attn[:, C + cs:C + cs + 512]).astype(bf),
            "wv": np.ascontiguousarray(w_attn[:, 2 * C + cs:2 * C + cs + 512]).astype(bf),
            "bqk": bqk,
            "bvr": np.ascontiguousarray(np.broadcast_to(bv[None, :], (128, 512))).astype(bf),
            "bpr": np.ascontiguousarray(np.broadcast_to(wpb[None, :], (128, C))).astype(bf),
            "masks": masks,
            "wp": np.ascontiguousarray(w_proj[cs:cs + 512, :]).astype(bf),
        })

    kw = {}
    if _trace:
        kw["trace"] = True
        if _trace_kwargs:
            kw.update(_trace_kwargs)
    res = run_bass_kernel_spmd(_get_nc(), in_maps, list(range(8)), **kw)
    _CACHE["last_results"] = res
    outs = [np.asarray(res.results[c]["out"], dtype=np.float32) for c in range(8)]
    y = np.stack([outs[2 * b] + outs[2 * b + 1] for b in range(B)])
    return y.astype(np.float32)


# revision 29
# speedup vs baseline: 1.0577x; 1.0547x over previous
"""Multi-head causal attention (B=4, T=2048, C=1024, H=16) on 8 trn2 cores.

Sharding: core = (batch b, head-half hg): each core computes QKV for batch b
and its 8 heads, causal flash-style attention (scores kept transposed
[key, query] so softmax denominators come from an appended ones-column in V),
and a partial output projection over its 512 y-features. Host sums the two
partial projections per batch (pure unshard-reduce; bias folded into the
hg==0 core's projection). No cross-core communication on device.

v2: phases are woven (P1 nt / P2 qt / P3 deferred) so the Tile scheduler
overlaps the scalar-engine exp backlog of attention with the QKV / output
projection matmuls; scalar engine runs exp only; diagonal score blocks are
query-range-restricted; output staged in bf16.
"""

import numpy as np
import ml_dtypes
import concourse.bass as bass
import concourse.mybir as mybir
import concourse.tile as tile
from concourse import bacc
from concourse.bass_utils import run_bass_kernel_spmd

B, T, C = 4, 2048, 1024
H, D = 16, 64
F32 = mybir.dt.float32
F32R = mybir.dt.float32r
BF16 = mybir.dt.bfloat16
AFT = mybir.ActivationFunctionType

_CACHE = {}


def build():
    nc = bacc.Bacc(None, target_bir_lowering=False)
    xt_d = nc.dram_tensor("xt", [C, T], BF16, kind="ExternalInput")
    wq_d = nc.dram_tensor("wq", [C, 512], BF16, kind="ExternalInput")
    wk_d = nc.dram_tensor("wk", [C, 512], BF16, kind="ExternalInput")
    wv_d = nc.dram_tensor("wv", [C, 512], BF16, kind="ExternalInput")
    bqk_d = nc.dram_tensor("bqk", [128, 8], F32, kind="ExternalInput")
    bvr_d = nc.dram_tensor("bvr", [128, 512], BF16, kind="ExternalInput")
    bpr_d = nc.dram_tensor("bpr", [128, C], BF16, kind="ExternalInput")
    masks_d = nc.dram_tensor("masks", [128, 896], BF16, kind="ExternalInput")
    wp_d = nc.dram_tensor("wp", [512, C], BF16, kind="ExternalInput")
    out_d = nc.dram_tensor("out", [T, C], BF16, kind="ExternalOutput")

    with nc.allow_low_precision(reason="fp32r matmul pipeline"):
        with tile.TileContext(nc) as tc:
            with (
                tc.tile_pool(name="const", bufs=1) as constp,
                tc.tile_pool(name="qk", bufs=1) as qkp,
                tc.tile_pool(name="vpool", bufs=1) as vp,
                tc.tile_pool(name="esb", bufs=2) as ep,
                tc.tile_pool(name="small", bufs=2) as smallp,
                tc.tile_pool(name="p1w", bufs=1) as p1wp,
                tc.tile_pool(name="p1x", bufs=1) as p1xp,
                tc.tile_pool(name="p3w", bufs=1) as p3wp,
                tc.tile_pool(name="ps", bufs=2, space="PSUM") as psp,
            ):
                bqk_t = constp.tile([128, 8], F32, tag="bqk")
                nc.sync.dma_start(bqk_t[:], bqk_d[:])
                maskE = constp.tile([128, 896], BF16, tag="maskE")
                nc.sync.dma_start(maskE[:], masks_d[:])

                qT = [qkp.tile([128, T], BF16, tag=f"qT{j}", name=f"qT{j}") for j in range(4)]
                kT = [qkp.tile([128, T], BF16, tag=f"kT{j}", name=f"kT{j}") for j in range(4)]
                vS = [vp.tile([128, 520], BF16, tag=f"v{t}", name=f"v{t}") for t in range(16)]
                yT = [qkp.tile([128, T], BF16, tag=f"yT{j}", name=f"yT{j}") for j in range(4)]

                # The whole startup is HBM-BW-bound and DMA-issue-rate-bound,
                # so each tensor is loaded with ONE multi-dim DMA, spread
                # across the three DMA-capable queues (sync/scalar/gpsimd),
                # x + wq (the first accumulation group's inputs) first.
                # Per-c 128-row DMAs (cheap ~600ns issues) into merged
                # tiles; nt0's x and wq are interleaved first so the first
                # accumulation group's c=0 inputs land ASAP.
                def load_x(nt, engs):
                    ts0 = nt * 512
                    xx = p1xp.tile([128, 4096], BF16, tag="xsb", bufs=2)
                    for c in range(8):
                        engs[c % len(engs)].dma_start(
                            xx[:, c * 512:(c + 1) * 512],
                            xt_d[c * 128:(c + 1) * 128, ts0:ts0 + 512])
                    return xx

                wq_sb = p1wp.tile([128, 4096], BF16, tag="wq")
                wk_sb = p1wp.tile([128, 4096], BF16, tag="wk")
                wv_sb = p1wp.tile([128, 4096], BF16, tag="wv")
                xt0 = p1xp.tile([128, 4096], BF16, tag="xsb", bufs=2)
                for c in range(8):
                    (nc.sync if c % 2 == 0 else nc.gpsimd).dma_start(
                        xt0[:, c * 512:(c + 1) * 512],
                        xt_d[c * 128:(c + 1) * 128, 0:512])
                    nc.scalar.dma_start(wq_sb[:, c * 512:(c + 1) * 512],
                                        wq_d[c * 128:(c + 1) * 128, :])
                for c in range(8):
                    (nc.gpsimd if c % 2 == 0 else nc.sync).dma_start(
                        wk_sb[:, c * 512:(c + 1) * 512],
                        wk_d[c * 128:(c + 1) * 128, :])
                for c in range(8):
                    (nc.sync if c % 2 == 0 else nc.gpsimd).dma_start(
                        wv_sb[:, c * 512:(c + 1) * 512],
                        wv_d[c * 128:(c + 1) * 128, :])
                bvr_t = p1wp.tile([128, 512], BF16, tag="bvr")
                nc.sync.dma_start(bvr_t[:], bvr_d[:])
                wp_sb = p3wp.tile([128, 4096], BF16, tag="wp")
                bpr_t = p3wp.tile([128, C], BF16, tag="bpr")
                onesc = constp.tile([1, 64], F32, tag="onesc")
                nc.vector.memset(onesc[:], 1.0)

                # ---------------- P1: QKV projections for token block nt ----
                def p1(nt, xt=None):
                    ts0 = nt * 512
                    if xt is None:
                        xt = load_x(nt, [nc.sync, nc.gpsimd])

                    def qk_group(ft):
                        q_ps = psp.tile([128, 512], F32, tag="mmps")
                        for c in range(8):
                            nc.tensor.matmul(q_ps[:],
                                             wq_sb[:, c * 512 + ft * 128:c * 512 + ft * 128 + 128],
                                             xt[:, c * 512:(c + 1) * 512],
                                             start=(c == 0), stop=(c == 7))
                        nc.vector.tensor_scalar_add(qT[ft][:, ts0:ts0 + 512], q_ps[:],
                                                    bqk_t[:, ft:ft + 1])
                        k_ps = psp.tile([128, 512], F32, tag="mmps")
                        for c in range(8):
                            nc.tensor.matmul(k_ps[:],
                                             wk_sb[:, c * 512 + ft * 128:c * 512 + ft * 128 + 128],
                                             xt[:, c * 512:(c + 1) * 512],
                                             start=(c == 0), stop=(c == 7))
                        nc.vector.tensor_scalar_add(kT[ft][:, ts0:ts0 + 512], k_ps[:],
                                                    bqk_t[:, 4 + ft:5 + ft])

                    def v_group(t2):
                        tt = nt * 4 + t2
                        v_ps = psp.tile([128, 512], F32, tag="mmps")
                        for c in range(8):
                            nc.tensor.matmul(v_ps[:],
                                             xt[:, c * 512 + t2 * 128:c * 512 + t2 * 128 + 128],
                                             wv_sb[:, c * 512:(c + 1) * 512],
                                             start=(c == 0), stop=(c == 7))
                        vv = vS[tt][:].rearrange("p (h c) -> p h c", c=65)
                        nc.vector.tensor_add(vv[:, :, 0:64],
                                             v_ps[:].rearrange("p (h c) -> p h c", c=64),
                                             bvr_t[:].rearrange("p (h c) -> p h c", c=64))
                        nc.vector.memset(vv[:, :, 64:65], 1.0)

                    if nt == 0:
                        # q/k head-pair 0 first, then all v, so attention on
                        # the first query block can start ASAP
                        qk_group(0)
                        for t2 in range(4):
                            v_group(t2)
                        for ft in range(1, 4):
                            qk_group(ft)
                    else:
                        for ft in range(4):
                            qk_group(ft)
                        for t2 in range(4):
                            v_group(t2)

                # ---------------- P2: attention for query block qt ----------
                def p2(qt):
                    q0 = qt * 512
                    ext = 4 * (qt + 1)
                    coll2 = [smallp.tile([8, 512], F32, tag=f"coll{a}", bufs=2,
                                         name=f"coll{a}") for a in range(2 if qt == 3 else 1)]
                    ysbs = []
                    for pj in range(4):
                        y_ps = [psp.tile([65, 512], F32, tag=f"yps{h}", bufs=1,
                                         name=f"yps{h}") for h in range(2)]
                        for sc in range(ext):
                            r = sc - (ext - 4)
                            qlo = 128 * r if r >= 2 else 0
                            s_ps = psp.tile([128, 1024], F32, tag="sps")
                            nc.tensor.matmul(s_ps[:, qlo:512],
                                             kT[pj][0:64, sc * 128:(sc + 1) * 128],
                                             qT[pj][0:64, q0 + qlo:q0 + 512],
                                             start=True, stop=True, tile_position=(0, 0))
                            nc.tensor.matmul(s_ps[:, 512 + qlo:1024],
                                             kT[pj][64:128, sc * 128:(sc + 1) * 128],
                                             qT[pj][64:128, q0 + qlo:q0 + 512],
                                             start=True, stop=True, tile_position=(64, 0))
                            e_t = ep.tile([128, 1024], BF16, tag="e", bufs=6)
                            if qlo == 0:
                                nc.scalar.activation(e_t[:], s_ps[:], AFT.Exp, scale=0.125)
                            else:
                                for h in range(2):
                                    nc.scalar.activation(
                                        e_t[:, 512 * h + qlo:512 * h + 512],
                                        s_ps[:, 512 * h + qlo:512 * h + 512],
                                        AFT.Exp, scale=0.125)
                            if r >= 0:
                                m0 = 384 - 128 * r
                                for h in range(2):
                                    nc.vector.tensor_mul(
                                        e_t[:, 512 * h + qlo:512 * h + 512],
                                        e_t[:, 512 * h + qlo:512 * h + 512],
                                        maskE[:, m0 + qlo:m0 + 512])
                            for h in range(2):
                                hc = 130 * pj + 65 * h
                                nc.tensor.matmul(y_ps[h][:, qlo:512], vS[sc][:, hc:hc + 65],
                                                 e_t[:, 512 * h + qlo:512 * h + 512],
                                                 start=(sc == 0), stop=(sc == ext - 1))
                        for h in range(2):
                            i = 2 * pj + h
                            y_sb = smallp.tile([65, 512], F32, tag="ysb", bufs=12)
                            nc.vector.tensor_copy(y_sb[:], y_ps[h][:])
                            if qt == 3 and pj >= 2:
                                nc.sync.dma_start(coll2[1][i - 4:i - 3, :], y_sb[64:65, :])
                            else:
                                nc.sync.dma_start(coll2[0][i:i + 1, :], y_sb[64:65, :])
                            ysbs.append(y_sb)
                    halves = [(0, 2, 0), (2, 4, 1)] if qt == 3 else [(0, 4, 0)]
                    for (plo, phi, a) in halves:
                        n2 = 2 * (phi - plo)
                        rec8 = smallp.tile([8, 512], F32, tag="rec8", bufs=2)
                        nc.vector.reciprocal_approx_fast(rec8[0:n2, :], coll2[a][0:n2, :])
                        for pj in range(plo, phi):
                            for h in range(2):
                                i = 2 * pj + h
                                i0 = i - 4 * a
                                r_t = smallp.tile([1, 512], F32, tag="rt", bufs=4)
                                nc.sync.dma_start(r_t[:], rec8[i0:i0 + 1, :])
                                rb_t = smallp.tile([64, 512], F32, tag="rbt", bufs=4)
                                nc.gpsimd.partition_broadcast(rb_t[:], r_t[:])
                                nc.vector.tensor_mul(yT[pj][64 * h:64 * h + 64, q0:q0 + 512],
                                                     ysbs[i][0:64, :], rb_t[:])

                # ---------------- P3: output projection for query block qt --
                def p3(qt):
                    for t2 in range(4):
                        tt = 4 * qt + t2
                        o_t = smallp.tile([128, 1024], BF16, tag="osb", bufs=4)
                        for of in range(2):
                            o_ps = psp.tile([128, 512], F32, tag="mmps")
                            for cy in range(4):
                                nc.tensor.matmul(
                                    o_ps[:],
                                    yT[cy][:, tt * 128:(tt + 1) * 128],
                                    wp_sb[:, cy * 1024 + of * 512:cy * 1024 + (of + 1) * 512],
                                    start=(cy == 0), stop=(cy == 3))
                            nc.vector.tensor_add(o_t[:, of * 512:(of + 1) * 512], o_ps[:],
                                                 bpr_t[:, of * 512:(of + 1) * 512])
                        nc.gpsimd.dma_start(out_d[tt * 128:(tt + 1) * 128, :], o_t[:])

                p1(0, xt0)
                for c in range(4):
                    nc.gpsimd.dma_start(wp_sb[:, c * 1024:(c + 1) * 1024],
                                        wp_d[c * 128:(c + 1) * 128, :])
                nc.sync.dma_start(bpr_t[:], bpr_d[:])
                for qt in range(4):
                    p2(qt)
                    if qt < 3:
                        p1(qt + 1)
                for qt in range(4):
                    p3(qt)

    if not nc.is_finalized():
        nc.finalize()
    return nc


def _get_nc():
    if "nc" not in _CACHE:
        _CACHE["nc"] = build()
    return _CACHE["nc"]


def _masks():
    i = np.arange(128)[:, None]
    x = np.arange(896)[None, :] - 384
    return np.where(i <= x, 1.0, 0.0).astype(ml_dtypes.bfloat16)


def kernel(x, w_attn, b_attn, w_proj, b_proj, _trace=False, _trace_kwargs=None):
    x = np.asarray(x, dtype=np.float32)
    w_attn = np.asarray(w_attn, dtype=np.float32)
    b_attn = np.asarray(b_attn, dtype=np.float32)
    w_proj = np.asarray(w_proj, dtype=np.float32)
    b_proj = np.asarray(b_proj, dtype=np.float32)

    masks = _masks()
    in_maps = []
    for core in range(8):
        b, hg = core // 2, core % 2
        cs = hg * 512
        bq = b_attn[cs:cs + 512]
        bk = b_attn[C + cs:C + cs + 512]
        bqk = np.concatenate([bq.reshape(4, 128).T, bk.reshape(4, 128).T],
                             axis=1).astype(np.float32)
        wpb = b_proj if hg == 0 else np.zeros_like(b_proj)
        bf = ml_dtypes.bfloat16
        bv = b_attn[2 * C + cs:2 * C + cs + 512]
        in_maps.append({
            "xt": np.ascontiguousarray(x[b].T).astype(bf),
            "wq": np.ascontiguousarray(w_attn[:, cs:cs + 512]).astype(bf),
            "wk": np.ascontiguousarray(w_attn[:, C + cs:C + cs + 512]).astype(bf),
            "wv": np.ascontiguousarray(w_attn[:, 2 * C + cs:2 * C + cs + 512]).astype(bf),
            "bqk": bqk,
            "bvr": np.ascontiguousarray(np.broadcast_to(bv[None, :], (128, 512))).astype(bf),
            "bpr": np.ascontiguousarray(np.broadcast_to(wpb[None, :], (128, C))).astype(bf),
            "masks": masks,
            "wp": np.ascontiguousarray(w_proj[cs:cs + 512, :]).astype(bf),
        })

    kw = {}
    if _trace:
        kw["trace"] = True
        if _trace_kwargs:
            kw.update(_trace_kwargs)
    res = run_bass_kernel_spmd(_get_nc(), in_maps, list(range(8)), **kw)
    _CACHE["last_results"] = res
    outs = [np.asarray(res.results[c]["out"], dtype=np.float32) for c in range(8)]
    y = np.stack([outs[2 * b] + outs[2 * b + 1] for b in range(B)])
    return y.astype(np.float32)


# revision 33
# speedup vs baseline: 1.0858x; 1.0265x over previous
"""Multi-head causal attention (B=4, T=2048, C=1024, H=16) on 8 trn2 cores.

Sharding: core = (batch b, head-half hg): each core computes QKV for batch b
and its 8 heads, causal flash-style attention (scores kept transposed
[key, query] so softmax denominators come from an appended ones-column in V),
and a partial output projection over its 512 y-features. Host sums the two
partial projections per batch (pure unshard-reduce; bias folded into the
hg==0 core's projection). No cross-core communication on device.

v2: phases are woven (P1 nt / P2 qt / P3 deferred) so the Tile scheduler
overlaps the scalar-engine exp backlog of attention with the QKV / output
projection matmuls; scalar engine runs exp only; diagonal score blocks are
query-range-restricted; output staged in bf16.
"""

import numpy as np
import ml_dtypes
import concourse.bass as bass
import concourse.mybir as mybir
import concourse.tile as tile
from concourse import bacc
from concourse.bass_utils import run_bass_kernel_spmd

B, T, C = 4, 2048, 1024
H, D = 16, 64
F32 = mybir.dt.float32
F32R = mybir.dt.float32r
BF16 = mybir.dt.bfloat16
AFT = mybir.ActivationFunctionType

_CACHE = {}


def build():
    nc = bacc.Bacc(None, target_bir_lowering=False)
    xt_d = nc.dram_tensor("xt", [C, T], BF16, kind="ExternalInput")
    wq_d = nc.dram_tensor("wq", [C, 512], BF16, kind="ExternalInput")
    wk_d = nc.dram_tensor("wk", [C, 512], BF16, kind="ExternalInput")
    wv_d = nc.dram_tensor("wv", [C, 512], BF16, kind="ExternalInput")
    bqk_d = nc.dram_tensor("bqk", [128, 8], F32, kind="ExternalInput")
    bvr_d = nc.dram_tensor("bvr", [128, 512], BF16, kind="ExternalInput")
    bpr_d = nc.dram_tensor("bpr", [128, C], BF16, kind="ExternalInput")
    masks_d = nc.dram_tensor("masks", [128, 896], BF16, kind="ExternalInput")
    wp_d = nc.dram_tensor("wp", [512, C], BF16, kind="ExternalInput")
    out_d = nc.dram_tensor("out", [T, C], BF16, kind="ExternalOutput")

    with nc.allow_low_precision(reason="fp32r matmul pipeline"):
        with tile.TileContext(nc) as tc:
            with (
                tc.tile_pool(name="const", bufs=1) as constp,
                tc.tile_pool(name="qk", bufs=1) as qkp,
                tc.tile_pool(name="vpool", bufs=1) as vp,
                tc.tile_pool(name="esb", bufs=2) as ep,
                tc.tile_pool(name="small", bufs=2) as smallp,
                tc.tile_pool(name="p1w", bufs=1) as p1wp,
                tc.tile_pool(name="p1x", bufs=1) as p1xp,
                tc.tile_pool(name="p3w", bufs=1) as p3wp,
                tc.tile_pool(name="ps", bufs=2, space="PSUM") as psp,
            ):
                bqk_t = constp.tile([128, 8], F32, tag="bqk")
                nc.sync.dma_start(bqk_t[:], bqk_d[:])
                maskE = constp.tile([128, 896], BF16, tag="maskE")
                nc.sync.dma_start(maskE[:], masks_d[:])

                qT = [qkp.tile([128, T], BF16, tag=f"qT{j}", name=f"qT{j}") for j in range(4)]
                kT = [qkp.tile([128, T], BF16, tag=f"kT{j}", name=f"kT{j}") for j in range(4)]
                vS = [vp.tile([128, 520], BF16, tag=f"v{t}", name=f"v{t}") for t in range(16)]
                yT = [qkp.tile([128, T], BF16, tag=f"yT{j}", name=f"yT{j}") for j in range(4)]

                # The whole startup is HBM-BW-bound and DMA-issue-rate-bound,
                # so each tensor is loaded with ONE multi-dim DMA, spread
                # across the three DMA-capable queues (sync/scalar/gpsimd),
                # x + wq (the first accumulation group's inputs) first.
                # Per-c 128-row DMAs (cheap ~600ns issues) into merged
                # tiles; nt0's x and wq are interleaved first so the first
                # accumulation group's c=0 inputs land ASAP.
                def load_x(nt, engs):
                    ts0 = nt * 512
                    xx = p1xp.tile([128, 4096], BF16, tag="xsb", bufs=2)
                    for c in range(8):
                        engs[c % len(engs)].dma_start(
                            xx[:, c * 512:(c + 1) * 512],
                            xt_d[c * 128:(c + 1) * 128, ts0:ts0 + 512])
                    return xx

                wq_sb = p1wp.tile([128, 4096], BF16, tag="wq")
                wk_sb = p1wp.tile([128, 4096], BF16, tag="wk")
                wv_sb = p1wp.tile([128, 4096], BF16, tag="wv")
                xt0 = p1xp.tile([128, 4096], BF16, tag="xsb", bufs=2)
                dma3 = [nc.sync, nc.gpsimd, nc.scalar]
                for c in range(8):
                    dma3[c % 3].dma_start(
                        xt0[:, c * 512:(c + 1) * 512],
                        xt_d[c * 128:(c + 1) * 128, 0:512])
                    dma3[(c + 1) % 3].dma_start(wq_sb[:, c * 512:(c + 1) * 512],
                                                wq_d[c * 128:(c + 1) * 128, :])
                for c in range(8):
                    (nc.gpsimd if c % 2 == 0 else nc.sync).dma_start(
                        wk_sb[:, c * 512:(c + 1) * 512],
                        wk_d[c * 128:(c + 1) * 128, :])
                for c in range(8):
                    (nc.sync if c % 2 == 0 else nc.gpsimd).dma_start(
                        wv_sb[:, c * 512:(c + 1) * 512],
                        wv_d[c * 128:(c + 1) * 128, :])
                bvr_t = p1wp.tile([128, 512], BF16, tag="bvr")
                nc.sync.dma_start(bvr_t[:], bvr_d[:])
                wp_sb = p3wp.tile([128, 4096], BF16, tag="wp")
                bpr_t = p3wp.tile([128, C], BF16, tag="bpr")
                onesc = constp.tile([1, 64], F32, tag="onesc")
                nc.vector.memset(onesc[:], 1.0)

                # ---------------- P1: QKV projections for token block nt ----
                def p1(nt, xt=None):
                    ts0 = nt * 512
                    if xt is None:
                        xt = load_x(nt, [nc.sync, nc.gpsimd])

                    def qk_group(ft):
                        q_ps = psp.tile([128, 512], F32, tag="mmps")
                        for c in range(8):
                            nc.tensor.matmul(q_ps[:],
                                             wq_sb[:, c * 512 + ft * 128:c * 512 + ft * 128 + 128],
                                             xt[:, c * 512:(c + 1) * 512],
                                             start=(c == 0), stop=(c == 7))
                        nc.vector.tensor_scalar_add(qT[ft][:, ts0:ts0 + 512], q_ps[:],
                                                    bqk_t[:, ft:ft + 1])
                        k_ps = psp.tile([128, 512], F32, tag="mmps")
                        for c in range(8):
                            nc.tensor.matmul(k_ps[:],
                                             wk_sb[:, c * 512 + ft * 128:c * 512 + ft * 128 + 128],
                                             xt[:, c * 512:(c + 1) * 512],
                                             start=(c == 0), stop=(c == 7))
                        nc.vector.tensor_scalar_add(kT[ft][:, ts0:ts0 + 512], k_ps[:],
                                                    bqk_t[:, 4 + ft:5 + ft])

                    def v_group(t2):
                        tt = nt * 4 + t2
                        v_ps = psp.tile([128, 512], F32, tag="mmps")
                        for c in range(8):
                            nc.tensor.matmul(v_ps[:],
                                             xt[:, c * 512 + t2 * 128:c * 512 + t2 * 128 + 128],
                                             wv_sb[:, c * 512:(c + 1) * 512],
                                             start=(c == 0), stop=(c == 7))
                        vv = vS[tt][:].rearrange("p (h c) -> p h c", c=65)
                        nc.vector.tensor_add(vv[:, :, 0:64],
                                             v_ps[:].rearrange("p (h c) -> p h c", c=64),
                                             bvr_t[:].rearrange("p (h c) -> p h c", c=64))
                        nc.vector.memset(vv[:, :, 64:65], 1.0)

                    if nt == 0:
                        # q/k head-pair 0 first, then all v, so attention on
                        # the first query block can start ASAP
                        qk_group(0)
                        for t2 in range(4):
                            v_group(t2)
                        for ft in range(1, 4):
                            qk_group(ft)
                    else:
                        for ft in range(4):
                            qk_group(ft)
                        for t2 in range(4):
                            v_group(t2)

                # ---------------- P2: attention for query block qt ----------
                def p2(qt):
                    q0 = qt * 512
                    ext = 4 * (qt + 1)
                    coll2 = [smallp.tile([8, 512], F32, tag=f"coll{a}", bufs=2,
                                         name=f"coll{a}") for a in range(1)]
                    ysbs = []
                    for pj in range(4):
                        y_ps = [psp.tile([65, 512], F32, tag=f"yps{h}", bufs=1,
                                         name=f"yps{h}") for h in range(2)]
                        for sc in range(ext):
                            r = sc - (ext - 4)
                            qlo = 128 * r if r >= 2 else 0
                            # r==1: exp covered by one call over cols
                            # [128, 1024) (h0's live range + all of h1)
                            qlo_h = [128 if r == 1 else qlo, qlo]
                            s_ps = psp.tile([128, 1024], F32, tag="sps")
                            nc.tensor.matmul(s_ps[:, qlo:512],
                                             kT[pj][0:64, sc * 128:(sc + 1) * 128],
                                             qT[pj][0:64, q0 + qlo:q0 + 512],
                                             start=True, stop=True, tile_position=(0, 0))
                            nc.tensor.matmul(s_ps[:, 512 + qlo:1024],
                                             kT[pj][64:128, sc * 128:(sc + 1) * 128],
                                             qT[pj][64:128, q0 + qlo:q0 + 512],
                                             start=True, stop=True, tile_position=(64, 0))
                            e_t = ep.tile([128, 1024], BF16, tag="e", bufs=6)
                            if r <= 0:
                                nc.scalar.activation(e_t[:], s_ps[:], AFT.Exp, scale=0.125)
                            elif r == 1:
                                nc.scalar.activation(e_t[:, 128:1024], s_ps[:, 128:1024],
                                                     AFT.Exp, scale=0.125)
                            else:
                                for h in range(2):
                                    nc.scalar.activation(
                                        e_t[:, 512 * h + qlo:512 * h + 512],
                                        s_ps[:, 512 * h + qlo:512 * h + 512],
                                        AFT.Exp, scale=0.125)
                            if r >= 0:
                                m0 = 384 - 128 * r
                                for h in range(2):
                                    nc.vector.tensor_mul(
                                        e_t[:, 512 * h + qlo_h[h]:512 * h + 512],
                                        e_t[:, 512 * h + qlo_h[h]:512 * h + 512],
                                        maskE[:, m0 + qlo_h[h]:m0 + 512])
                            for h in range(2):
                                hc = 130 * pj + 65 * h
                                ql = qlo_h[h]
                                nc.tensor.matmul(y_ps[h][:, ql:512], vS[sc][:, hc:hc + 65],
                                                 e_t[:, 512 * h + ql:512 * h + 512],
                                                 start=(sc == 0), stop=(sc == ext - 1))
                        for h in range(2):
                            i = 2 * pj + h
                            y_sb = smallp.tile([65, 512], F32, tag="ysb", bufs=12)
                            if qt == 3 and pj >= 2:
                                # tail-critical: copy on the (drained) scalar
                                # engine; denominators handled per-tile below
                                nc.scalar.copy(y_sb[:], y_ps[h][:])
                            else:
                                nc.vector.tensor_copy(y_sb[:], y_ps[h][:])
                                nc.sync.dma_start(coll2[0][i:i + 1, :], y_sb[64:65, :])
                            ysbs.append(y_sb)
                    for (plo, phi, a) in [(0, 4, 0)] if qt < 3 else [(0, 2, 0)]:
                        n2 = 2 * (phi - plo)
                        rec8 = smallp.tile([8, 512], F32, tag="rec8", bufs=2)
                        nc.vector.reciprocal_approx_fast(rec8[0:n2, :], coll2[a][0:n2, :])
                        for pj in range(plo, phi):
                            for h in range(2):
                                i = 2 * pj + h
                                r_t = smallp.tile([1, 512], F32, tag="rt", bufs=6)
                                nc.sync.dma_start(r_t[:], rec8[i:i + 1, :])
                                rb_t = smallp.tile([64, 512], F32, tag="rbt", bufs=6)
                                nc.gpsimd.partition_broadcast(rb_t[:], r_t[:])
                                nc.vector.tensor_mul(yT[pj][64 * h:64 * h + 64, q0:q0 + 512],
                                                     ysbs[i][0:64, :], rb_t[:])
                    if qt == 3:
                        for pj in range(2, 4):
                            for h in range(2):
                                i = 2 * pj + h
                                r_t = smallp.tile([1, 512], F32, tag="rt", bufs=6)
                                nc.sync.dma_start(r_t[:], ysbs[i][64:65, :])
                                rr_t = smallp.tile([1, 512], F32, tag="rrt", bufs=4)
                                nc.vector.reciprocal_approx_fast(rr_t[:], r_t[:])
                                rb_t = smallp.tile([64, 512], F32, tag="rbt", bufs=6)
                                nc.gpsimd.partition_broadcast(rb_t[:], rr_t[:])
                                nc.vector.tensor_mul(yT[pj][64 * h:64 * h + 64, q0:q0 + 512],
                                                     ysbs[i][0:64, :], rb_t[:])

                # ---------------- P3: output projection for query block qt --
                def p3(qt):
                    for t2 in range(4):
                        tt = 4 * qt + t2
                        o_t = smallp.tile([128, 1024], BF16, tag="osb", bufs=4)
                        for of in range(2):
                            o_ps = psp.tile([128, 512], F32, tag="mmps")
                            for cy in range(4):
                                nc.tensor.matmul(
                                    o_ps[:],
                                    yT[cy][:, tt * 128:(tt + 1) * 128],
                                    wp_sb[:, cy * 1024 + of * 512:cy * 1024 + (of + 1) * 512],
                                    start=(cy == 0), stop=(cy == 3))
                            nc.vector.tensor_add(o_t[:, of * 512:(of + 1) * 512], o_ps[:],
                                                 bpr_t[:, of * 512:(of + 1) * 512])
                        nc.gpsimd.dma_start(out_d[tt * 128:(tt + 1) * 128, :], o_t[:])

                p1(0, xt0)
                for c in range(4):
                    nc.gpsimd.dma_start(wp_sb[:, c * 1024:(c + 1) * 1024],
                                        wp_d[c * 128:(c + 1) * 128, :])
                nc.sync.dma_start(bpr_t[:], bpr_d[:])
                for qt in range(4):
                    p2(qt)
                    if qt < 3:
                        p1(qt + 1)
                for qt in range(4):
                    p3(qt)

    if not nc.is_finalized():
        nc.finalize()
    return nc


def _get_nc():
    if "nc" not in _CACHE:
        _CACHE["nc"] = build()
    return _CACHE["nc"]


def _masks():
    i = np.arange(128)[:, None]
    x = np.arange(896)[None, :] - 384
    return np.where(i <= x, 1.0, 0.0).astype(ml_dtypes.bfloat16)


def kernel(x, w_attn, b_attn, w_proj, b_proj, _trace=False, _trace_kwargs=None):
    x = np.asarray(x, dtype=np.float32)
    w_attn = np.asarray(w_attn, dtype=np.float32)
    b_attn = np.asarray(b_attn, dtype=np.float32)
    w_proj = np.asarray(w_proj, dtype=np.float32)
    b_proj = np.asarray(b_proj, dtype=np.float32)

    masks = _masks()
    in_maps = []
    for core in range(8):
        b, hg = core // 2, core % 2
        cs = hg * 512
        bq = b_attn[cs:cs + 512]
        bk = b_attn[C + cs:C + cs + 512]
        bqk = np.concatenate([bq.reshape(4, 128).T, bk.reshape(4, 128).T],
                             axis=1).astype(np.float32)
        wpb = b_proj if hg == 0 else np.zeros_like(b_proj)
        bf = ml_dtypes.bfloat16
        bv = b_attn[2 * C + cs:2 * C + cs + 512]
        in_maps.append({
            "xt": np.ascontiguousarray(x[b].T).astype(bf),
            "wq": np.ascontiguousarray(w_attn[:, cs:cs + 512]).astype(bf),
            "wk": np.ascontiguousarray(w_attn[:, C + cs:C + cs + 512]).astype(bf),
            "wv": np.ascontiguousarray(w_attn[:, 2 * C + cs:2 * C + cs + 512]).astype(bf),
            "bqk": bqk,
            "bvr": np.ascontiguousarray(np.broadcast_to(bv[None, :], (128, 512))).astype(bf),
            "bpr": np.ascontiguousarray(np.broadcast_to(wpb[None, :], (128, C))).astype(bf),
            "masks": masks,
            "wp": np.ascontiguousarray(w_proj[cs:cs + 512, :]).astype(bf),
        })

    kw = {}
    if _trace:
        kw["trace"] = True
        if _trace_kwargs:
            kw.update(_trace_kwargs)
    res = run_bass_kernel_spmd(_get_nc(), in_maps, list(range(8)), **kw)
    _CACHE["last_results"] = res
    outs = [np.asarray(res.results[c]["out"], dtype=np.float32) for c in range(8)]
    y = np.stack([outs[2 * b] + outs[2 * b + 1] for b in range(B)])
    return y.astype(np.float32)
